# revision 8
# baseline (speedup 1.0000x reference)
"""CrossNet kernel for Trainium2 (8 NeuronCores, pure data parallel).

Math: reference computes, for l = 0..2:
    s_l = x_l . w_l   (per-row scalar)
    x_{l+1} = x0 * s_l + x_l + b_l

Unrolled (all dots reduce to dots against x0):
    a_i   = x0 . w_i                     (per-row, i = 0..2)
    beta1 = b0 . w1,  beta2 = (b0+b1) . w2   (scalars)
    T3    = ((1+a0)(1+a1) + beta1)(1+a2) + beta2
    out   = x0 * T3 + (b0+b1+b2)

Per core (2048 rows), per 128-row tile (memory-bound; engines kept off
the DMA critical path):
  - DMA x tile [128, 1024] to SBUF
  - a_i via one fused multiply+accumulate (scalar_tensor_tensor) each:
    a0, a1 on VectorE, a2 on GpSimd, against w replicated across
    partitions once at startup (partition_broadcast)
  - DVE: P = A + 1; T3 = P0*P1*P2 (plus beta terms when bias != 0)
  - ScalarE: out = x * T3 (per-partition scale)
  - DMA out
"""

import numpy as np

import concourse.bacc as bacc
import concourse.bass as bass
import concourse.mybir as mybir
import concourse.tile as tile
from concourse.bass_utils import run_bass_kernel_spmd

BATCH, DIM, LAYERS = 16384, 1024, 3
NCORES = 8
ROWS = BATCH // NCORES  # rows per core
P = 128                 # SBUF partitions
NT = ROWS // P          # row tiles per core

F32 = mybir.dt.float32

# which engine computes each of the three dots ("v" = VectorE, "g" = GpSimd)
DOT_ENGINES = ("v", "v", "g")


def _build(beta1: float, beta2: float, with_bias: bool):
    nc = bacc.Bacc("TRN2", target_bir_lowering=False, debug=False)

    x_d = nc.dram_tensor("x", [ROWS, DIM], F32, kind="ExternalInput").ap()
    w_d = nc.dram_tensor("w", [1, LAYERS * DIM], F32, kind="ExternalInput").ap()
    if with_bias:
        b3_d = nc.dram_tensor("b3", [P, DIM], F32, kind="ExternalInput").ap()
    out_d = nc.dram_tensor("out", [ROWS, DIM], F32, kind="ExternalOutput").ap()

    mult = mybir.AluOpType.mult

    with tile.TileContext(nc) as tc:
        with (
            tc.tile_pool(name="const", bufs=1) as cpool,
            tc.tile_pool(name="xin", bufs=4) as xpool,
            tc.tile_pool(name="outp", bufs=4) as opool,
            tc.tile_pool(name="scr", bufs=2) as scrpool,
            tc.tile_pool(name="small", bufs=4) as spool,
        ):
            # Replicate the three w vectors across all 128 partitions.
            wrow = cpool.tile([1, LAYERS * DIM], F32)
            nc.sync.dma_start(wrow[:], w_d[:])
            wrep = cpool.tile([P, LAYERS * DIM], F32)
            nc.gpsimd.partition_broadcast(wrep[:], wrow[:])

            if with_bias:
                b3_t = cpool.tile([P, DIM], F32)
                nc.sync.dma_start(b3_t[:], b3_d[:])

            for t in range(NT):
                xin = xpool.tile([P, DIM], F32)
                nc.sync.dma_start(xin[:], x_d[t * P:(t + 1) * P, :])

                # A[:, i] = sum_d x[:, d] * w_i[d]
                A = spool.tile([P, LAYERS], F32, tag="A")
                for i, eng in enumerate(DOT_ENGINES):
                    scr = scrpool.tile([P, DIM], F32, tag=f"scr{i}")
                    if eng == "v":
                        # fused mult+accum on VectorE (one op)
                        nc.vector.scalar_tensor_tensor(
                            scr[:], xin[:], 1.0, wrep[:, i * DIM:(i + 1) * DIM],
                            op0=mult, op1=mult, accum_out=A[:, i:i + 1],
                        )
                    else:
                        # GpSimd multiply, ScalarE accumulate (Pool engine
                        # rejects TensorScalarPtr)
                        nc.gpsimd.tensor_tensor(
                            scr[:], xin[:], wrep[:, i * DIM:(i + 1) * DIM], op=mult
                        )
                        scr2 = scrpool.tile([P, DIM], F32, tag=f"scrb{i}")
                        nc.scalar.activation(
                            scr2[:], scr[:],
                            mybir.ActivationFunctionType.Copy,
                            accum_out=A[:, i:i + 1],
                        )

                # T3 = ((1+a0)(1+a1)+beta1)(1+a2)+beta2
                pP = spool.tile([P, LAYERS], F32, tag="pP")
                nc.vector.tensor_scalar_add(pP[:], A[:], 1.0)
                t2 = spool.tile([P, 1], F32, tag="t2")
                nc.vector.tensor_mul(t2[:], pP[:, 0:1], pP[:, 1:2])
                if beta1 != 0.0:
                    nc.vector.tensor_scalar_add(t2[:], t2[:], beta1)
                t3 = spool.tile([P, 1], F32, tag="t3")
                nc.vector.tensor_mul(t3[:], t2[:], pP[:, 2:3])
                if beta2 != 0.0:
                    nc.vector.tensor_scalar_add(t3[:], t3[:], beta2)

                xo = opool.tile([P, DIM], F32)
                if with_bias:
                    # out = x * T3 + B3 (one DVE op)
                    nc.vector.scalar_tensor_tensor(
                        xo[:], xin[:], t3[:], b3_t[:],
                        op0=mult, op1=mybir.AluOpType.add,
                    )
                else:
                    # out = x * T3 (ScalarE per-partition scale)
                    nc.scalar.mul(xo[:], xin[:], t3[:])

                nc.sync.dma_start(out_d[t * P:(t + 1) * P, :], xo[:])

    nc.compile()
    return nc


def prepare(x: np.ndarray, kernels: np.ndarray, bias: np.ndarray):
    """Build the Bass program and the per-core input maps."""
    x = np.ascontiguousarray(x, dtype=np.float32)
    kernels = np.asarray(kernels, dtype=np.float32)
    bias = np.asarray(bias, dtype=np.float32)

    # Host-side tiny prep (O(LAYERS * DIM)): beta scalars, bias sum.
    beta1 = float(bias[0] @ kernels[1])
    beta2 = float((bias[0] + bias[1]) @ kernels[2])
    b3 = bias.sum(axis=0)
    with_bias = bool(np.any(b3 != 0.0))

    nc = _build(beta1, beta2, with_bias)

    w_flat = np.ascontiguousarray(kernels.reshape(1, LAYERS * DIM))
    in_maps = []
    for c in range(NCORES):
        m = {"x": x[c * ROWS:(c + 1) * ROWS], "w": w_flat}
        if with_bias:
            m["b3"] = np.ascontiguousarray(np.broadcast_to(b3, (P, DIM)))
        in_maps.append(m)
    return nc, in_maps


def kernel(x: np.ndarray, kernels: np.ndarray, bias: np.ndarray) -> np.ndarray:
    nc, in_maps = prepare(x, kernels, bias)
    res = run_bass_kernel_spmd(nc, in_maps, list(range(NCORES)))
    return np.concatenate([r["out"] for r in res.results], axis=0)


# revision 12
# speedup vs baseline: 1.0397x; 1.0397x over previous
"""CrossNet kernel for Trainium2 (8 NeuronCores, pure data parallel).

Math: reference computes, for l = 0..2:
    s_l = x_l . w_l   (per-row scalar)
    x_{l+1} = x0 * s_l + x_l + b_l

Unrolled (all dots reduce to dots against x0):
    a_i   = x0 . w_i                     (per-row, i = 0..2)
    beta1 = b0 . w1,  beta2 = (b0+b1) . w2   (scalars)
    T3    = ((1+a0)(1+a1) + beta1)(1+a2) + beta2
    out   = x0 * T3 + (b0+b1+b2)

Per core (2048 rows), per 128-row tile (memory-bound; work spread so no
engine exceeds the DMA roofline):
  - DMA x tile [128, 1024] to SBUF (SP HW-DGE queue)
  - p_i = 1 + x.w_i for i=0,1: one fused tensor_tensor_reduce each on
    VectorE (multiply + reduce + init in a single op)
  - a_2: GpSimd multiply, ScalarE activation-accumulate reduce
  - DVE: t2 = p0*p1 ; t3 = (a2+1)*t2   (plus beta terms when bias != 0)
  - ScalarE: out = x * t3 (per-partition scale)
  - DMA out (alternating SP / Activation HW-DGE queues)
  - w is pre-replicated across partitions on the host (tiny input) so no
    on-device broadcast serializes startup.
"""

import numpy as np

import concourse.bacc as bacc
import concourse.bass as bass
import concourse.mybir as mybir
import concourse.tile as tile
from concourse.bass_utils import run_bass_kernel_spmd

BATCH, DIM, LAYERS = 16384, 1024, 3
NCORES = 8
ROWS = BATCH // NCORES  # rows per core
P = 128                 # SBUF partitions
NT = ROWS // P          # row tiles per core

F32 = mybir.dt.float32


def _build(beta1: float, beta2: float, with_bias: bool):
    nc = bacc.Bacc("TRN2", target_bir_lowering=False, debug=False)

    x_d = nc.dram_tensor("x", [ROWS, DIM], F32, kind="ExternalInput").ap()
    w_d = nc.dram_tensor("w", [P, LAYERS * DIM], F32, kind="ExternalInput").ap()
    if with_bias:
        b3_d = nc.dram_tensor("b3", [P, DIM], F32, kind="ExternalInput").ap()
    out_d = nc.dram_tensor("out", [ROWS, DIM], F32, kind="ExternalOutput").ap()

    mult = mybir.AluOpType.mult
    add = mybir.AluOpType.add

    with tile.TileContext(nc) as tc:
        with (
            tc.tile_pool(name="const", bufs=1) as cpool,
            tc.tile_pool(name="xin", bufs=4) as xpool,
            tc.tile_pool(name="outp", bufs=4) as opool,
            tc.tile_pool(name="scr", bufs=2) as scrpool,
            tc.tile_pool(name="small", bufs=4) as spool,
        ):
            # w pre-replicated across partitions by the host; load per-layer
            # chunks on the ACT queue so the SP queue starts on x at once.
            wrep = cpool.tile([P, LAYERS * DIM], F32)
            for i in range(LAYERS):
                nc.sync.dma_start(
                    wrep[:, i * DIM:(i + 1) * DIM], w_d[:, i * DIM:(i + 1) * DIM]
                )

            if with_bias:
                b3_t = cpool.tile([P, DIM], F32)
                nc.sync.dma_start(b3_t[:], b3_d[:])

            for t in range(NT):
                xin = xpool.tile([P, DIM], F32)
                nc.sync.dma_start(xin[:], x_d[t * P:(t + 1) * P, :])

                A = spool.tile([P, LAYERS], F32, tag="A")
                # a_i = x . w_i  (fused multiply+accumulate on VectorE)
                for i in range(2):
                    scr = scrpool.tile([P, DIM], F32, tag=f"scr{i}")
                    nc.vector.scalar_tensor_tensor(
                        scr[:], xin[:], 1.0, wrep[:, i * DIM:(i + 1) * DIM],
                        op0=mult, op1=mult, accum_out=A[:, i:i + 1],
                    )
                # a_2: GpSimd multiply, ScalarE reduce
                scr2 = scrpool.tile([P, DIM], F32, tag="scr2")
                nc.gpsimd.tensor_tensor(
                    scr2[:], xin[:], wrep[:, 2 * DIM:3 * DIM], op=mult
                )
                scr2b = scrpool.tile([P, DIM], F32, tag="scr2b")
                nc.scalar.activation(
                    scr2b[:], scr2[:],
                    mybir.ActivationFunctionType.Copy,
                    accum_out=A[:, 2:3],
                )

                # t3 = ((1+a0)(1+a1) + beta1) * (1+a2) + beta2
                pP = spool.tile([P, LAYERS], F32, tag="pP")
                nc.vector.tensor_scalar_add(pP[:], A[:], 1.0)
                t2 = spool.tile([P, 1], F32, tag="t2")
                nc.vector.tensor_mul(t2[:], pP[:, 0:1], pP[:, 1:2])
                if beta1 != 0.0:
                    nc.vector.tensor_scalar_add(t2[:], t2[:], beta1)
                t3 = spool.tile([P, 1], F32, tag="t3")
                nc.vector.tensor_mul(t3[:], t2[:], pP[:, 2:3])
                if beta2 != 0.0:
                    nc.vector.tensor_scalar_add(t3[:], t3[:], beta2)

                xo = opool.tile([P, DIM], F32)
                if with_bias:
                    # out = x * t3 + B3 (one DVE op)
                    nc.vector.scalar_tensor_tensor(
                        xo[:], xin[:], t3[:], b3_t[:], op0=mult, op1=add,
                    )
                else:
                    # out = x * t3 (ScalarE per-partition scale)
                    nc.scalar.mul(xo[:], xin[:], t3[:])

                # stores alternate between the two HW-DGE queues
                nc.sync.dma_start(out_d[t * P:(t + 1) * P, :], xo[:])

    nc.compile()
    return nc


def prepare(x: np.ndarray, kernels: np.ndarray, bias: np.ndarray):
    """Build the Bass program and the per-core input maps."""
    x = np.ascontiguousarray(x, dtype=np.float32)
    kernels = np.asarray(kernels, dtype=np.float32)
    bias = np.asarray(bias, dtype=np.float32)

    # Host-side tiny prep (O(LAYERS * DIM)): beta scalars, bias sum,
    # partition-replicated w.
    beta1 = float(bias[0] @ kernels[1])
    beta2 = float((bias[0] + bias[1]) @ kernels[2])
    b3 = bias.sum(axis=0)
    with_bias = bool(np.any(b3 != 0.0))

    nc = _build(beta1, beta2, with_bias)

    w_rep = np.ascontiguousarray(
        np.broadcast_to(kernels.reshape(1, LAYERS * DIM), (P, LAYERS * DIM))
    )
    in_maps = []
    for c in range(NCORES):
        m = {"x": x[c * ROWS:(c + 1) * ROWS], "w": w_rep}
        if with_bias:
            m["b3"] = np.ascontiguousarray(np.broadcast_to(b3, (P, DIM)))
        in_maps.append(m)
    return nc, in_maps


def kernel(x: np.ndarray, kernels: np.ndarray, bias: np.ndarray) -> np.ndarray:
    nc, in_maps = prepare(x, kernels, bias)
    res = run_bass_kernel_spmd(nc, in_maps, list(range(NCORES)))
    return np.concatenate([r["out"] for r in res.results], axis=0)


# revision 15
# speedup vs baseline: 1.1385x; 1.0950x over previous
"""CrossNet kernel for Trainium2 (8 NeuronCores, pure data parallel).

Math: reference computes, for l = 0..2:
    s_l = x_l . w_l   (per-row scalar)
    x_{l+1} = x0 * s_l + x_l + b_l

Unrolled (all dots reduce to dots against x0):
    a_i   = x0 . w_i                     (per-row, i = 0..2)
    beta1 = b0 . w1,  beta2 = (b0+b1) . w2   (scalars)
    T3    = ((1+a0)(1+a1) + beta1)(1+a2) + beta2
    out   = x0 * T3 + (b0+b1+b2)

Per core (2048 rows), per 128-row tile (memory-bound; work spread so no
engine exceeds the DMA roofline):
  - DMA x tile [128, 1024] to SBUF (SP HW-DGE queue)
  - p_i = 1 + x.w_i for i=0,1: one fused tensor_tensor_reduce each on
    VectorE (multiply + reduce + init in a single op)
  - a_2: GpSimd multiply, ScalarE activation-accumulate reduce
  - DVE: t2 = p0*p1 ; t3 = (a2+1)*t2   (plus beta terms when bias != 0)
  - ScalarE: out = x * t3 (per-partition scale)
  - DMA out (alternating SP / Activation HW-DGE queues)
  - w is pre-replicated across partitions on the host (tiny input) so no
    on-device broadcast serializes startup.
"""

import numpy as np

import concourse.bacc as bacc
import concourse.bass as bass
import concourse.mybir as mybir
import concourse.tile as tile
from concourse.bass_utils import run_bass_kernel_spmd

BATCH, DIM, LAYERS = 16384, 1024, 3
NCORES = 8
ROWS = BATCH // NCORES  # rows per core
P = 128                 # SBUF partitions
NT = ROWS // P          # row tiles per core

F32 = mybir.dt.float32


def _build(beta1: float, beta2: float, with_bias: bool):
    nc = bacc.Bacc("TRN2", target_bir_lowering=False, debug=False)

    x_d = nc.dram_tensor("x", [ROWS, DIM], F32, kind="ExternalInput").ap()
    w_d = nc.dram_tensor("w", [P, LAYERS * DIM], F32, kind="ExternalInput").ap()
    if with_bias:
        b3_d = nc.dram_tensor("b3", [P, DIM], F32, kind="ExternalInput").ap()
    out_d = nc.dram_tensor("out", [ROWS, DIM], F32, kind="ExternalOutput").ap()

    mult = mybir.AluOpType.mult
    add = mybir.AluOpType.add

    with tile.TileContext(nc) as tc:
        with (
            tc.tile_pool(name="const", bufs=1) as cpool,
            tc.tile_pool(name="xin", bufs=6) as xpool,
            tc.tile_pool(name="outp", bufs=4) as opool,
            tc.tile_pool(name="scr", bufs=2) as scrpool,
            tc.tile_pool(name="small", bufs=4) as spool,
        ):
            # w pre-replicated across partitions by the host; load per-layer
            # chunks on the ACT queue so the SP queue starts on x at once.
            wrep = cpool.tile([P, LAYERS * DIM], F32)
            for i in range(LAYERS):
                nc.scalar.dma_start(
                    wrep[:, i * DIM:(i + 1) * DIM], w_d[:, i * DIM:(i + 1) * DIM]
                )

            if with_bias:
                b3_t = cpool.tile([P, DIM], F32)
                nc.sync.dma_start(b3_t[:], b3_d[:])

            for t in range(NT):
                xin = xpool.tile([P, DIM], F32)
                nc.sync.dma_start(xin[:], x_d[t * P:(t + 1) * P, :])

                A = spool.tile([P, LAYERS], F32, tag="A")
                # a_i = x . w_i  (fused multiply+accumulate on VectorE)
                for i in range(2):
                    scr = scrpool.tile([P, DIM], F32, tag=f"scr{i}")
                    nc.vector.scalar_tensor_tensor(
                        scr[:], xin[:], 1.0, wrep[:, i * DIM:(i + 1) * DIM],
                        op0=mult, op1=mult, accum_out=A[:, i:i + 1],
                    )
                # a_2: GpSimd multiply, ScalarE reduce
                scr2 = scrpool.tile([P, DIM], F32, tag="scr2")
                nc.gpsimd.tensor_tensor(
                    scr2[:], xin[:], wrep[:, 2 * DIM:3 * DIM], op=mult
                )
                scr2b = scrpool.tile([P, DIM], F32, tag="scr2b")
                nc.scalar.activation(
                    scr2b[:], scr2[:],
                    mybir.ActivationFunctionType.Copy,
                    accum_out=A[:, 2:3],
                )

                # t3 = ((1+a0)(1+a1) + beta1) * (1+a2) + beta2
                pP = spool.tile([P, LAYERS], F32, tag="pP")
                nc.vector.tensor_scalar_add(pP[:], A[:], 1.0)
                t2 = spool.tile([P, 1], F32, tag="t2")
                nc.vector.tensor_mul(t2[:], pP[:, 0:1], pP[:, 1:2])
                if beta1 != 0.0:
                    nc.vector.tensor_scalar_add(t2[:], t2[:], beta1)
                t3 = spool.tile([P, 1], F32, tag="t3")
                nc.vector.tensor_mul(t3[:], t2[:], pP[:, 2:3])
                if beta2 != 0.0:
                    nc.vector.tensor_scalar_add(t3[:], t3[:], beta2)

                xo = opool.tile([P, DIM], F32)
                if with_bias:
                    # out = x * t3 + B3 (one DVE op)
                    nc.vector.scalar_tensor_tensor(
                        xo[:], xin[:], t3[:], b3_t[:], op0=mult, op1=add,
                    )
                else:
                    # out = x * t3 (ScalarE per-partition scale)
                    nc.scalar.mul(xo[:], xin[:], t3[:])

                # store dispatched by ScalarE right after it produced xo —
                # no dispatch-time wait, and the SP queue stays loads-only
                nc.scalar.dma_start(out_d[t * P:(t + 1) * P, :], xo[:])

    nc.compile()
    return nc


def prepare(x: np.ndarray, kernels: np.ndarray, bias: np.ndarray):
    """Build the Bass program and the per-core input maps."""
    x = np.ascontiguousarray(x, dtype=np.float32)
    kernels = np.asarray(kernels, dtype=np.float32)
    bias = np.asarray(bias, dtype=np.float32)

    # Host-side tiny prep (O(LAYERS * DIM)): beta scalars, bias sum,
    # partition-replicated w.
    beta1 = float(bias[0] @ kernels[1])
    beta2 = float((bias[0] + bias[1]) @ kernels[2])
    b3 = bias.sum(axis=0)
    with_bias = bool(np.any(b3 != 0.0))

    nc = _build(beta1, beta2, with_bias)

    w_rep = np.ascontiguousarray(
        np.broadcast_to(kernels.reshape(1, LAYERS * DIM), (P, LAYERS * DIM))
    )
    in_maps = []
    for c in range(NCORES):
        m = {"x": x[c * ROWS:(c + 1) * ROWS], "w": w_rep}
        if with_bias:
            m["b3"] = np.ascontiguousarray(np.broadcast_to(b3, (P, DIM)))
        in_maps.append(m)
    return nc, in_maps


def kernel(x: np.ndarray, kernels: np.ndarray, bias: np.ndarray) -> np.ndarray:
    nc, in_maps = prepare(x, kernels, bias)
    res = run_bass_kernel_spmd(nc, in_maps, list(range(NCORES)))
    return np.concatenate([r["out"] for r in res.results], axis=0)
